# revision 2
# baseline (speedup 1.0000x reference)
"""Multi-head self-attention Trainium2 kernel (8-core data parallel).

Reference computation (per batch b):
  q/k/v = einsum('sd,hda->hsa', x[b], W[:,i])       i in {0,1,2}
  scores = q @ k^T / sqrt(64); probs = softmax(scores)
  out = probs @ v; cat = concat heads [s, h*a]; z = cat @ Wz

Strategy per core (1 batch per core):
  - host pre-transposes x[b] -> xT [d, s] and flattens W head-major, all bf16
  - qT,kT computed W-stationary: qT[ha, s] tiles (2 heads per 128-partition tile)
  - v computed x-stationary in natural [t, ha] layout, stored interleaved with a
    ones column per head: v_sb[t, h, 0:64]=v, v_sb[t, h, 64]=1
  - scoresT[t, s] = kT.T @ qT per head (K=64); exp on ScalarE with scale=1/8,
    no max subtraction (|scores/8| <~ 5.5, safely in fp32/bf16 exp range)
  - out^T accumulated via lhsT=[v|1]: psum rows 0..63 = v^T @ expT (unnormalized),
    row 64 = sum_t expT = softmax denominator
  - normalize: reciprocal of row 64, DMA-broadcast across 64 partitions,
    multiply -> catT[ha, s] bf16 (exactly the lhsT layout the final matmul wants)
  - zT[64, s] = Wz.T @ catT; PE-transpose to z[s, 64] fp32 and DMA out
"""

import sys

sys.path.insert(0, "/opt/trn_rl_repo")

import numpy as np
import ml_dtypes

import concourse.bass as bass
import concourse.bacc as bacc
import concourse.tile as tile
import concourse.mybir as mybir
from concourse.bass_utils import run_bass_kernel_spmd
from concourse.masks import make_identity

F32 = mybir.dt.float32
BF16 = mybir.dt.bfloat16
BF = ml_dtypes.bfloat16

S = 1024  # sequence length
D = 1024  # model dim
H = 16    # heads
A = 64    # attention dim per head
B = 8     # batch (one per core)
NT = 8    # 128-row tiles per 1024 dim

TRACE = False
LAST_EXEC_NS = None

_PROGRAM = None


def _build_program():
    nc = bacc.Bacc("TRN2", target_bir_lowering=False, debug=False)

    xT = nc.dram_tensor("xT", [D, S], BF16, kind="ExternalInput").ap()
    wq = nc.dram_tensor("wq", [D, H * A], BF16, kind="ExternalInput").ap()
    wk = nc.dram_tensor("wk", [D, H * A], BF16, kind="ExternalInput").ap()
    wv = nc.dram_tensor("wv", [D, H * A], BF16, kind="ExternalInput").ap()
    wz = nc.dram_tensor("wz", [H * A, A], BF16, kind="ExternalInput").ap()
    out = nc.dram_tensor("out", [S, A], F32, kind="ExternalOutput").ap()

    with tile.TileContext(nc) as tc:
        with (
            tc.tile_pool(name="persist", bufs=1) as pers,
            tc.tile_pool(name="probs", bufs=2) as ppool,
            tc.tile_pool(name="small", bufs=2) as small,
            tc.tile_pool(name="psbig", bufs=2, space="PSUM") as psbig,
            tc.tile_pool(name="pspv", bufs=2, space="PSUM") as pspv,
        ):
            xt_sb = pers.tile([128, NT, S], BF16)
            wq_sb = pers.tile([128, NT, H * A], BF16)
            wk_sb = pers.tile([128, NT, H * A], BF16)
            wv_sb = pers.tile([128, NT, H * A], BF16)
            wz_sb = pers.tile([128, NT, A], BF16)
            v_sb = pers.tile([128, NT, H, A + 1], BF16)
            qt_sb = pers.tile([128, NT, S], BF16)
            kt_sb = pers.tile([128, NT, S], BF16)
            catt_sb = pers.tile([128, NT, S], BF16)
            ident = pers.tile([64, 64], F32)
            zt_sb = pers.tile([64, S], F32)
            out_sb = pers.tile([128, NT, A], F32)

            # input DMAs (per 128-row tile for fine-grained deps)
            for d in range(NT):
                r = slice(d * 128, (d + 1) * 128)
                nc.sync.dma_start(out=xt_sb[:, d, :], in_=xT[r, :])
                nc.sync.dma_start(out=wv_sb[:, d, :], in_=wv[r, :])
                nc.sync.dma_start(out=wq_sb[:, d, :], in_=wq[r, :])
                nc.sync.dma_start(out=wk_sb[:, d, :], in_=wk[r, :])
                nc.sync.dma_start(out=wz_sb[:, d, :], in_=wz[r, :])

            # ones column per head for the softmax denominator row
            nc.vector.memset(v_sb[:, :, :, A : A + 1], 1.0)
            make_identity(nc, ident)

            # ---- V (natural [t, ha] layout, x-stationary) ----
            for tt in range(NT):
                pv = psbig.tile([128, 1024], F32, tag="big")
                for d in range(NT):
                    for nh in range(2):
                        nc.tensor.matmul(
                            pv[:, nh * 512 : (nh + 1) * 512],
                            xt_sb[:, d, tt * 128 : (tt + 1) * 128],
                            wv_sb[:, d, nh * 512 : (nh + 1) * 512],
                            start=(d == 0),
                            stop=(d == NT - 1),
                        )
                nc.vector.tensor_copy(
                    out=v_sb[:, tt, :, 0:A],
                    in_=pv[:].rearrange("p (h a) -> p h a", h=H),
                )

            # ---- Q^T, K^T (W-stationary, 2 heads per tile) ----
            for hp in range(NT):
                for which, w_sb, dst in (("q", wq_sb, qt_sb), ("k", wk_sb, kt_sb)):
                    pq = psbig.tile([128, 1024], F32, tag="big")
                    for d in range(NT):
                        for sh in range(2):
                            nc.tensor.matmul(
                                pq[:, sh * 512 : (sh + 1) * 512],
                                w_sb[:, d, hp * 128 : (hp + 1) * 128],
                                xt_sb[:, d, sh * 512 : (sh + 1) * 512],
                                start=(d == 0),
                                stop=(d == NT - 1),
                            )
                    nc.vector.tensor_copy(out=dst[:, hp, :], in_=pq[:])

            # ---- attention per head ----
            for h in range(H):
                hp = h // 2
                po = (h % 2) * 64
                probs = ppool.tile([128, NT, S], BF16, tag="probs")
                for tt in range(NT):
                    ps = psbig.tile([128, 1024], F32, tag="big")
                    for sh in range(2):
                        nc.tensor.matmul(
                            ps[:, sh * 512 : (sh + 1) * 512],
                            kt_sb[po : po + 64, hp, tt * 128 : (tt + 1) * 128],
                            qt_sb[po : po + 64, hp, sh * 512 : (sh + 1) * 512],
                            start=True,
                            stop=True,
                        )
                    nc.scalar.activation(
                        out=probs[:, tt, :],
                        in_=ps[:],
                        func=mybir.ActivationFunctionType.Exp,
                        scale=0.125,
                    )
                po_ps = pspv.tile([A + 1, 1024], F32, tag="pv")
                for tt in range(NT):
                    for sh in range(2):
                        nc.tensor.matmul(
                            po_ps[:, sh * 512 : (sh + 1) * 512],
                            v_sb[:, tt, h, :],
                            probs[:, tt, sh * 512 : (sh + 1) * 512],
                            start=(tt == 0),
                            stop=(tt == NT - 1),
                        )
                recip = small.tile([1, S], F32, tag="recip")
                nc.vector.reciprocal(out=recip[:], in_=po_ps[A : A + 1, :])
                bc = small.tile([64, S], F32, tag="bc")
                nc.gpsimd.partition_broadcast(bc[:], recip[:])
                nc.vector.tensor_mul(catt_sb[po : po + 64, hp, :], po_ps[0:A, :], bc[:])

            # ---- final projection z^T = Wz^T @ catT ----
            pz = psbig.tile([128, 1024], F32, tag="big")
            for kt in range(NT):
                for sh in range(2):
                    nc.tensor.matmul(
                        pz[0:A, sh * 512 : (sh + 1) * 512],
                        wz_sb[:, kt, :],
                        catt_sb[:, kt, sh * 512 : (sh + 1) * 512],
                        start=(kt == 0),
                        stop=(kt == NT - 1),
                    )
            nc.vector.tensor_copy(out=zt_sb[:], in_=pz[0:A, :])

            # transpose zT [64, s] -> z [s, 64] via PE, 128 rows at a time
            for st in range(NT):
                pt = psbig.tile([128, 1024], F32, tag="big")
                nc.tensor.transpose(
                    pt[:, 0:A], zt_sb[:, st * 128 : (st + 1) * 128], ident[:]
                )
                nc.vector.tensor_copy(out=out_sb[:, st, :], in_=pt[:, 0:A])

            nc.sync.dma_start(
                out=out.rearrange("(st p) n -> p st n", p=128), in_=out_sb[:]
            )

    nc.compile()
    return nc


def _get_program():
    global _PROGRAM
    if _PROGRAM is None:
        _PROGRAM = _build_program()
    return _PROGRAM


def kernel(x: np.ndarray, W: np.ndarray, Wz: np.ndarray) -> np.ndarray:
    global LAST_EXEC_NS
    assert x.shape == (B, S, D) and W.shape == (H, 3, D, A) and Wz.shape == (H * A, A)

    # host-side prep: flatten weights head-major [d, h*a], cast to bf16
    Wf = W.astype(BF)
    wq_h = np.ascontiguousarray(Wf[:, 0].transpose(1, 0, 2).reshape(D, H * A))
    wk_h = np.ascontiguousarray(Wf[:, 1].transpose(1, 0, 2).reshape(D, H * A))
    wv_h = np.ascontiguousarray(Wf[:, 2].transpose(1, 0, 2).reshape(D, H * A))
    wz_h = np.ascontiguousarray(Wz.astype(BF))

    in_maps = []
    for b in range(B):
        xt = np.ascontiguousarray(x[b].T.astype(BF))
        in_maps.append({"xT": xt, "wq": wq_h, "wk": wk_h, "wv": wv_h, "wz": wz_h})

    nc = _get_program()
    res = run_bass_kernel_spmd(nc, in_maps, core_ids=list(range(B)), trace=TRACE)
    LAST_EXEC_NS = res.exec_time_ns
    return np.stack([res.results[b]["out"] for b in range(B)], axis=0)


# revision 4
# speedup vs baseline: 1.4058x; 1.4058x over previous
"""Multi-head self-attention Trainium2 kernel (8-core data parallel).

Reference computation (per batch b):
  q/k/v = einsum('sd,hda->hsa', x[b], W[:,i])       i in {0,1,2}
  scores = q @ k^T / sqrt(64); probs = softmax(scores)
  out = probs @ v; cat = concat heads [s, h*a]; z = cat @ Wz

Strategy per core (1 batch per core):
  - host pre-transposes x[b] -> xT [d, s] and flattens W head-major, all bf16
  - qT,kT computed W-stationary: qT[ha, s] tiles (2 heads per 128-partition tile)
  - v computed x-stationary in natural [t, ha] layout, stored interleaved with a
    ones column per head: v_sb[t, h, 0:64]=v, v_sb[t, h, 64]=1
  - scoresT[t, s] = kT.T @ qT per head (K=64); exp on ScalarE with scale=1/8,
    no max subtraction (|scores/8| <~ 5.5, safely in fp32/bf16 exp range)
  - out^T accumulated via lhsT=[v|1]: psum rows 0..63 = v^T @ expT (unnormalized),
    row 64 = sum_t expT = softmax denominator
  - normalize: reciprocal of row 64, DMA-broadcast across 64 partitions,
    multiply -> catT[ha, s] bf16 (exactly the lhsT layout the final matmul wants)
  - zT[64, s] = Wz.T @ catT; PE-transpose to z[s, 64] fp32 and DMA out
"""

import sys

sys.path.insert(0, "/opt/trn_rl_repo")

import numpy as np
import ml_dtypes

import concourse.bass as bass
import concourse.bacc as bacc
import concourse.tile as tile
import concourse.mybir as mybir
from concourse.bass_utils import run_bass_kernel_spmd
from concourse.masks import make_identity

F32 = mybir.dt.float32
BF16 = mybir.dt.bfloat16
BF = ml_dtypes.bfloat16

S = 1024  # sequence length
D = 1024  # model dim
H = 16    # heads
A = 64    # attention dim per head
B = 8     # batch (one per core)
NT = 8    # 128-row tiles per 1024 dim

TRACE = False
LAST_EXEC_NS = None

_PROGRAM = None


def _build_program():
    nc = bacc.Bacc("TRN2", target_bir_lowering=False, debug=False)

    xT = nc.dram_tensor("xT", [D, S], BF16, kind="ExternalInput").ap()
    wq = nc.dram_tensor("wq", [D, H * A], BF16, kind="ExternalInput").ap()
    wk = nc.dram_tensor("wk", [D, H * A], BF16, kind="ExternalInput").ap()
    wv = nc.dram_tensor("wv", [D, H * A], BF16, kind="ExternalInput").ap()
    wz = nc.dram_tensor("wz", [H * A, A], BF16, kind="ExternalInput").ap()
    out = nc.dram_tensor("out", [S, A], F32, kind="ExternalOutput").ap()

    with tile.TileContext(nc) as tc:
        with (
            tc.tile_pool(name="persist", bufs=1) as pers,
            tc.tile_pool(name="probs", bufs=2) as ppool,
            tc.tile_pool(name="small", bufs=2) as small,
            tc.tile_pool(name="psbig", bufs=2, space="PSUM") as psbig,
            tc.tile_pool(name="pspv", bufs=2, space="PSUM") as pspv,
        ):
            xt_sb = pers.tile([128, NT, S], BF16)
            wq_sb = pers.tile([128, NT, H * A], BF16)
            wk_sb = pers.tile([128, NT, H * A], BF16)
            wv_sb = pers.tile([128, NT, H * A], BF16)
            wz_sb = pers.tile([128, NT, A], BF16)
            v_sb = pers.tile([128, NT, H, A + 1], BF16)
            qt_sb = pers.tile([128, NT, S], BF16)
            kt_sb = pers.tile([128, NT, S], BF16)
            catt_sb = pers.tile([128, NT, S], BF16)
            ident = pers.tile([64, 64], F32)
            zt_sb = pers.tile([64, S], F32)
            out_sb = pers.tile([128, NT, A], F32)

            # input DMAs (per 128-row tile for fine-grained deps)
            for d in range(NT):
                r = slice(d * 128, (d + 1) * 128)
                nc.sync.dma_start(out=xt_sb[:, d, :], in_=xT[r, :])
                nc.sync.dma_start(out=wv_sb[:, d, :], in_=wv[r, :])
                nc.sync.dma_start(out=wq_sb[:, d, :], in_=wq[r, :])
                nc.sync.dma_start(out=wk_sb[:, d, :], in_=wk[r, :])
                nc.sync.dma_start(out=wz_sb[:, d, :], in_=wz[r, :])

            # ones column per head for the softmax denominator row
            nc.vector.memset(v_sb[:, :, :, A : A + 1], 1.0)
            make_identity(nc, ident)

            # ---- V (natural [t, ha] layout, x-stationary) ----
            for tt in range(NT):
                pv = psbig.tile([128, 1024], F32, tag="big")
                for d in range(NT):
                    for nh in range(2):
                        nc.tensor.matmul(
                            pv[:, nh * 512 : (nh + 1) * 512],
                            xt_sb[:, d, tt * 128 : (tt + 1) * 128],
                            wv_sb[:, d, nh * 512 : (nh + 1) * 512],
                            start=(d == 0),
                            stop=(d == NT - 1),
                        )
                nc.vector.tensor_copy(
                    out=v_sb[:, tt, :, 0:A],
                    in_=pv[:].rearrange("p (h a) -> p h a", h=H),
                )

            # ---- Q^T, K^T (W-stationary, 2 heads per tile) ----
            for hp in range(NT):
                for which, w_sb, dst in (("q", wq_sb, qt_sb), ("k", wk_sb, kt_sb)):
                    pq = psbig.tile([128, 1024], F32, tag="big")
                    for d in range(NT):
                        for sh in range(2):
                            nc.tensor.matmul(
                                pq[:, sh * 512 : (sh + 1) * 512],
                                w_sb[:, d, hp * 128 : (hp + 1) * 128],
                                xt_sb[:, d, sh * 512 : (sh + 1) * 512],
                                start=(d == 0),
                                stop=(d == NT - 1),
                            )
                    nc.vector.tensor_copy(out=dst[:, hp, :], in_=pq[:])

            # ---- attention per head ----
            for h in range(H):
                hp = h // 2
                po = (h % 2) * 64
                probs = ppool.tile([128, NT, S], BF16, tag="probs")
                for tt in range(NT):
                    ps = psbig.tile([128, 1024], F32, tag="big")
                    for sh in range(2):
                        nc.tensor.matmul(
                            ps[:, sh * 512 : (sh + 1) * 512],
                            kt_sb[po : po + 64, hp, tt * 128 : (tt + 1) * 128],
                            qt_sb[po : po + 64, hp, sh * 512 : (sh + 1) * 512],
                            start=True,
                            stop=True,
                        )
                    nc.scalar.activation(
                        out=probs[:, tt, :],
                        in_=ps[:],
                        func=mybir.ActivationFunctionType.Exp,
                        scale=0.125,
                    )
                po_ps = pspv.tile([A + 1, 1024], F32, tag="pv")
                for tt in range(NT):
                    for sh in range(2):
                        nc.tensor.matmul(
                            po_ps[:, sh * 512 : (sh + 1) * 512],
                            v_sb[:, tt, h, :],
                            probs[:, tt, sh * 512 : (sh + 1) * 512],
                            start=(tt == 0),
                            stop=(tt == NT - 1),
                        )
                den = small.tile([1, S], F32, tag="den")
                nc.vector.tensor_copy(out=den[:], in_=po_ps[A : A + 1, :])
                recip = small.tile([1, S], F32, tag="recip")
                nc.vector.reciprocal_approx_fast(out=recip[:], in_=den[:])
                bc = small.tile([64, S], F32, tag="bc")
                nc.gpsimd.partition_broadcast(bc[:], recip[:])
                nc.vector.tensor_mul(catt_sb[po : po + 64, hp, :], po_ps[0:A, :], bc[:])

            # ---- final projection z^T = Wz^T @ catT ----
            pz = psbig.tile([128, 1024], F32, tag="big")
            for kt in range(NT):
                for sh in range(2):
                    nc.tensor.matmul(
                        pz[0:A, sh * 512 : (sh + 1) * 512],
                        wz_sb[:, kt, :],
                        catt_sb[:, kt, sh * 512 : (sh + 1) * 512],
                        start=(kt == 0),
                        stop=(kt == NT - 1),
                    )
            nc.vector.tensor_copy(out=zt_sb[:], in_=pz[0:A, :])

            # transpose zT [64, s] -> z [s, 64] via PE, 128 rows at a time
            for st in range(NT):
                pt = psbig.tile([128, 1024], F32, tag="big")
                nc.tensor.transpose(
                    pt[:, 0:A], zt_sb[:, st * 128 : (st + 1) * 128], ident[:]
                )
                nc.vector.tensor_copy(out=out_sb[:, st, :], in_=pt[:, 0:A])

            nc.sync.dma_start(
                out=out.rearrange("(st p) n -> p st n", p=128), in_=out_sb[:]
            )

    nc.compile()
    return nc


def _get_program():
    global _PROGRAM
    if _PROGRAM is None:
        _PROGRAM = _build_program()
    return _PROGRAM


def kernel(x: np.ndarray, W: np.ndarray, Wz: np.ndarray) -> np.ndarray:
    global LAST_EXEC_NS
    assert x.shape == (B, S, D) and W.shape == (H, 3, D, A) and Wz.shape == (H * A, A)

    # host-side prep: flatten weights head-major [d, h*a], cast to bf16
    Wf = W.astype(BF)
    wq_h = np.ascontiguousarray(Wf[:, 0].transpose(1, 0, 2).reshape(D, H * A))
    wk_h = np.ascontiguousarray(Wf[:, 1].transpose(1, 0, 2).reshape(D, H * A))
    wv_h = np.ascontiguousarray(Wf[:, 2].transpose(1, 0, 2).reshape(D, H * A))
    wz_h = np.ascontiguousarray(Wz.astype(BF))

    in_maps = []
    for b in range(B):
        xt = np.ascontiguousarray(x[b].T.astype(BF))
        in_maps.append({"xT": xt, "wq": wq_h, "wk": wk_h, "wv": wv_h, "wz": wz_h})

    nc = _get_program()
    res = run_bass_kernel_spmd(nc, in_maps, core_ids=list(range(B)), trace=TRACE)
    LAST_EXEC_NS = res.exec_time_ns
    return np.stack([res.results[b]["out"] for b in range(B)], axis=0)
